# revision 33
# baseline (speedup 1.0000x reference)
"""AttentionPooling (segment softmax-pool) TRN2 kernel, 8-core SPMD.

Self-contained: kernel(**inputs) -> np.ndarray [16384, 128] f32.

Math (shift-invariance of softmax; logits are O(1) so exp can't overflow):
  e_i   = exp(tanh(x_i @ W1 + b1) @ W2 + b2)
  out_g = (sum_{i in g} e_i x_i) / (sum_{i in g} e_i)

Sharding: graphs are split into 8 contiguous ranges with ~equal node counts
(each graph's nodes land on one core); each core computes its own rows of the
output; host concatenates.

Device algorithm per core: x is streamed in two bf16 layouts (natural tiles
for the pooling matmul, transposed for the MLP matmul), batched into 4MB/2MB
DMA transfers. Per 4096-row chunk: 8 PE matmuls (W1) + 8 512-wide tanh
activations, 32 N=1 logit matmuls, one exp; the masked-e matrix for all 32
tiles is built with two batched DVE tensor_tensor ops (is_equal + mult
against broadcast slot/e columns); 32 pooling matmuls accumulate
masked_e^T @ [x|1] into PSUM. A static bf16 indicator matmul re-bins
window-slots to segments and DVE reciprocal+scale normalizes.

Scheduling: a three-stage software pipeline interleaved per cycle —
logit MMs of chunk c-1 run at cycle start (exp fires mid-cycle), dense MLP
matmuls are spread across the cycle (keeps the PE HAM activity monitor at
full clock), and pool matmuls of chunk c-3 fill the remaining PE slots.
Measured: 444892 ns HW exec (from 717790 ns baseline), rel err 6.9e-3.
"""

import math

import numpy as np
import ml_dtypes

BF16 = ml_dtypes.bfloat16

N_CORES = 8
N_GRAPHS = 16384
H = 128
TILE = 128
TPC = 32             # tiles per window
CHUNK = TILE * TPC   # 4096 rows
SLOTS = 64           # max segments per window
NW_STITCH = 8        # stitch window span (static)
NCHK = NW_STITCH * SLOTS // 128
MSLOT = 44           # active slot width (real data max is 36/window)
PAD_SEG = 9999.0

LAST_EXEC_NS = None
_PROGRAM_CACHE = {}


# ---------------------------------------------------------------- host prep
def _preprocess(x, batch, W1, b1, W2, b2, n_graphs):
    N = x.shape[0]
    counts = np.bincount(batch, minlength=n_graphs)
    cum = np.zeros(n_graphs + 1, dtype=np.int64)
    np.cumsum(counts, out=cum[1:])

    gsplit = [0]
    for c in range(1, N_CORES):
        t = round(c * N / N_CORES)
        g = int(np.searchsorted(cum, t))
        if g > 0 and abs(cum[g - 1] - t) <= abs(cum[g] - t):
            g -= 1
        g = max(g, gsplit[-1] + 1)
        gsplit.append(min(g, n_graphs - (N_CORES - c)))
    gsplit.append(n_graphs)
    gsplit = np.array(gsplit, dtype=np.int64)

    Mc = [int(cum[gsplit[c + 1]] - cum[gsplit[c]]) for c in range(N_CORES)]
    Gc = [int(gsplit[c + 1] - gsplit[c]) for c in range(N_CORES)]
    NWIN = max(NW_STITCH, math.ceil(max(Mc) / CHUNK))
    if NWIN % 2:
        NWIN += 1
    M_pad = NWIN * CHUNK
    NGRP = math.ceil(max(Gc) / 128)

    x = np.asarray(x, dtype=np.float32)
    batch = np.asarray(batch)

    cores = []
    minw = np.full((N_CORES, NGRP), 10 ** 9, dtype=np.int64)
    maxw = np.full((N_CORES, NGRP), -1, dtype=np.int64)
    for c in range(N_CORES):
        nlo = int(cum[gsplit[c]])
        nhi = int(cum[gsplit[c + 1]])
        m = Mc[c]
        bl = batch[nlo:nhi].astype(np.int64) - gsplit[c]
        wfs = np.zeros(NWIN, dtype=np.int64)
        for w in range(NWIN):
            wfs[w] = bl[w * CHUNK] if w * CHUNK < m else Gc[c]
        slots = bl - wfs[np.arange(m) // CHUNK]
        assert slots.min() >= 0 and slots.max() < MSLOT, (
            f"core {c}: window slot range {slots.min()}..{slots.max()}")

        seg = np.full(M_pad, PAD_SEG, dtype=np.float32)
        seg[:m] = slots.astype(np.float32)
        seg_img = np.ascontiguousarray(seg.reshape(-1, TILE).T.astype(BF16))

        nT = M_pad // TILE
        xn = np.zeros((M_pad, H + 1), dtype=BF16)
        xn[:m, :H] = x[nlo:nhi]
        xn[:, H] = 1.0
        xn_img = np.ascontiguousarray(
            xn.reshape(nT, TILE, H + 1).transpose(1, 0, 2).reshape(TILE, nT * (H + 1)))

        xt = np.zeros((M_pad, H), dtype=BF16)
        xt[:m] = x[nlo:nhi]
        xt_img = np.ascontiguousarray(xt.T)

        lo_g = cum[gsplit[c]:gsplit[c + 1]] - nlo
        hi_g = cum[gsplit[c] + 1:gsplit[c + 1] + 1] - nlo
        nonempty = hi_g > lo_g
        wlo_g = np.where(nonempty, lo_g // CHUNK, 0)
        whi_g = np.where(nonempty, np.maximum(hi_g - 1, 0) // CHUNK, 0)
        for Gi in range(NGRP):
            a, b = Gi * 128, min(Gi * 128 + 128, Gc[c])
            if a >= Gc[c]:
                continue
            ne = nonempty[a:b]
            if ne.any():
                minw[c, Gi] = wlo_g[a:b][ne].min()
                maxw[c, Gi] = whi_g[a:b][ne].max()
        cores.append(dict(m=m, gc=Gc[c], wfs=wfs, seg_img=seg_img,
                          xn_img=xn_img, xt_img=xt_img, nonempty=nonempty,
                          wlo_g=wlo_g, whi_g=whi_g))

    wlo_shared = []
    for Gi in range(NGRP):
        mn = int(minw[:, Gi].min())
        if mn >= 10 ** 9:
            mn = 0
        mn -= mn % 2
        mn = max(0, min(mn, NWIN - NW_STITCH))
        wlo_shared.append(mn)
        mx = int(maxw[:, Gi].max())
        assert mx < 0 or mx - mn + 1 <= NW_STITCH, (
            f"group {Gi}: window span {mn}..{mx} exceeds {NW_STITCH}")

    for c in range(N_CORES):
        d = cores[c]
        ind = np.zeros((128, NGRP * NCHK * 128), dtype=BF16)
        for g in range(d["gc"]):
            if not d["nonempty"][g]:
                continue
            Gi = g // 128
            base_ws = wlo_shared[Gi] * SLOTS
            for w in range(int(d["wlo_g"][g]), int(d["whi_g"][g]) + 1):
                s = int(g - d["wfs"][w])
                wsl = w * SLOTS + s - base_ws
                assert 0 <= wsl < NW_STITCH * SLOTS
                ind[wsl % 128, (Gi * NCHK + wsl // 128) * 128 + (g - Gi * 128)] = 1.0
        d["ind_img"] = ind

    shared = dict(
        NWIN=NWIN, M_pad=M_pad, NGRP=NGRP, wlo_shared=wlo_shared,
        gsplit=gsplit, counts=counts,
        iota=np.ascontiguousarray(
            np.tile(np.arange(MSLOT, dtype=BF16), TPC).reshape(1, TPC * MSLOT)
            * np.ones((128, 1), dtype=BF16)),
        w1b=np.ascontiguousarray(np.asarray(W1).astype(BF16)),
        w2b=np.ascontiguousarray(np.asarray(W2).astype(BF16)),
        b1c=np.ascontiguousarray(np.asarray(b1).reshape(H, 1).astype(np.float32)),
        b2c=np.full((128, 1), np.asarray(b2).reshape(-1)[0], dtype=np.float32),
    )
    return shared, cores


# ---------------------------------------------------------------- program
def _build_program(NWIN, NGRP, wlo_shared,
                   xt_bufs=2, xn_bufs=3):
    from contextlib import ExitStack
    import concourse.bacc as bacc
    import concourse.tile as tile
    from concourse import mybir

    M_pad = NWIN * CHUNK
    nT = M_pad // TILE
    NWCOL = NWIN * SLOTS // 128

    f32 = mybir.dt.float32
    bf16 = mybir.dt.bfloat16
    AF = mybir.ActivationFunctionType
    ALU = mybir.AluOpType

    nc = bacc.Bacc("TRN2", target_bir_lowering=False, debug=False,
                   enable_asserts=False, num_devices=N_CORES)
    xt_ap = nc.dram_tensor("xT", [128, M_pad], bf16, kind="ExternalInput").ap()
    xn_ap = nc.dram_tensor("xn", [128, nT * (H + 1)], bf16, kind="ExternalInput").ap()
    seg_ap = nc.dram_tensor("seg", [128, nT], bf16, kind="ExternalInput").ap()
    iota_ap = nc.dram_tensor("iota", [128, TPC * MSLOT], bf16,
                             kind="ExternalInput").ap()
    w1_ap = nc.dram_tensor("w1b", [128, H], bf16, kind="ExternalInput").ap()
    w2_ap = nc.dram_tensor("w2b", [128, 1], bf16, kind="ExternalInput").ap()
    b1_ap = nc.dram_tensor("b1c", [128, 1], f32, kind="ExternalInput").ap()
    b2_ap = nc.dram_tensor("b2c", [128, 1], f32, kind="ExternalInput").ap()
    ind_ap = nc.dram_tensor("ind", [128, NGRP * NCHK * 128], bf16,
                            kind="ExternalInput").ap()
    out_ap = nc.dram_tensor("out", [NGRP * 128, H], f32, kind="ExternalOutput").ap()

    with tile.TileContext(nc) as tc, ExitStack() as ctx:
        consts = ctx.enter_context(tc.tile_pool(name="consts", bufs=1))
        xt_pool = ctx.enter_context(tc.tile_pool(name="xt", bufs=xt_bufs))
        xn_pool = ctx.enter_context(tc.tile_pool(name="xnp", bufs=4))
        seg_pool = ctx.enter_context(tc.tile_pool(name="segp", bufs=xn_bufs))
        h_pool = ctx.enter_context(tc.tile_pool(name="hp", bufs=18))
        ee_pool = ctx.enter_context(tc.tile_pool(name="ep", bufs=4))
        me01_pool = ctx.enter_context(tc.tile_pool(name="me01p", bufs=3))
        me_pool = ctx.enter_context(tc.tile_pool(name="mep", bufs=4))
        wres_pool = ctx.enter_context(tc.tile_pool(name="wres", bufs=1))
        ind_pool = ctx.enter_context(tc.tile_pool(name="indp", bufs=4))
        r_pool = ctx.enter_context(tc.tile_pool(name="rp", bufs=2))
        ob_pool = ctx.enter_context(tc.tile_pool(name="obp", bufs=2))
        ht_psum = ctx.enter_context(tc.tile_pool(name="htps", bufs=5, space="PSUM"))
        lg_psum = ctx.enter_context(tc.tile_pool(name="lgps", bufs=1, space="PSUM"))
        pl_psum = ctx.enter_context(tc.tile_pool(name="plps", bufs=2, space="PSUM"))

        iota_t = consts.tile([128, TPC * MSLOT], bf16, tag="iota")
        nc.sync.dma_start(iota_t[:], iota_ap[:])
        w1_t = consts.tile([128, H], bf16, tag="w1")
        nc.sync.dma_start(w1_t[:], w1_ap[:])
        w2_t = consts.tile([128, 1], bf16, tag="w2")
        nc.sync.dma_start(w2_t[:], w2_ap[:])
        b1_t = consts.tile([128, 1], f32, tag="b1")
        nc.sync.dma_start(b1_t[:], b1_ap[:])
        b2_t = consts.tile([128, 1], f32, tag="b2")
        nc.sync.dma_start(b2_t[:], b2_ap[:])
        wres_cols = [wres_pool.tile([128, H + 1], bf16, name=f"wres{i}",
                                    tag=f"wres{i}")
                     for i in range(NWCOL)]
        for i in range(NWCOL):
            nc.vector.memset(wres_cols[i][:], 0.0)

        # emit group Gi's stitch right after its last window is flushed
        ready_groups = {}
        for Gi in range(NGRP):
            ready_groups.setdefault(wlo_shared[Gi] + NW_STITCH - 1, []).append(Gi)

        def emit_stitch(Gi):
            st = pl_psum.tile([128, H + 1], f32, tag="pl")
            it = ind_pool.tile([128, NCHK * 128], bf16)
            nc.sync.dma_start(
                it[:], ind_ap[:, Gi * NCHK * 128:(Gi + 1) * NCHK * 128])
            for k in range(NCHK):
                wc = wlo_shared[Gi] // 2 + k
                nc.tensor.matmul(st[:], lhsT=it[:, k * 128:(k + 1) * 128],
                                 rhs=wres_cols[wc][:],
                                 start=(k == 0), stop=(k == NCHK - 1))
            r = r_pool.tile([128, 1], f32)
            nc.vector.reciprocal(r[:], st[:, H:H + 1])
            ob = ob_pool.tile([128, H], f32)
            nc.vector.tensor_scalar(ob[:], st[:, 0:H], r[:, 0:1], None,
                                    op0=ALU.mult)
            nc.sync.dma_start(out_ap[Gi * 128:(Gi + 1) * 128, :], ob[:])

        # batched DMA transfers: xT in 4-chunk groups (freed right after the
        # MLP MMs, so 2 bufs give ~4 chunks of lead), xn in 2-chunk groups
        # (lives until the lag-2 pool, so 3 bufs).
        DBT, DBN = 4, 2
        xt_grp = {}
        xn_grp = {}

        def chunk_setup(c):
            if c % DBT == 0 and c not in xt_grp:
                n = min(DBT, NWIN - c)
                xt4 = xt_pool.tile([128, n * CHUNK], bf16, name=f"xt{c}", tag="xt")
                nc.sync.dma_start(xt4[:], xt_ap[:, c * CHUNK:(c + n) * CHUNK])
                xt_grp[c] = xt4
            if c % DBN == 0 and c not in xn_grp:
                n = min(DBN, NWIN - c)
                xn2 = xn_pool.tile([128, n * TPC * (H + 1)], bf16,
                                   name=f"xn{c}", tag="xn")
                nc.scalar.dma_start(
                    xn2[:], xn_ap[:, c * TPC * (H + 1):(c + n) * TPC * (H + 1)])
                sg2 = seg_pool.tile([128, n * TPC], bf16, name=f"sg{c}", tag="sg")
                nc.sync.dma_start(sg2[:], seg_ap[:, c * TPC:(c + n) * TPC])
                xn_grp[c] = (xn2, sg2)
            kt, kn = c % DBT, c % DBN
            xt = xt_grp[c - kt][:, kt * CHUNK:(kt + 1) * CHUNK]
            xn2, sg2 = xn_grp[c - kn]
            xn = xn2[:, kn * TPC * (H + 1):(kn + 1) * TPC * (H + 1)]
            sg = sg2[:, kn * TPC:(kn + 1) * TPC]
            return xt, xn, sg

        def mlp_mm(c, xt, q):
            ht = ht_psum.tile([128, 512], f32, name=f"ht{c}_{q}", tag="ht")
            nc.tensor.matmul(ht[:], lhsT=w1_t[:],
                             rhs=xt[:, q * 512:(q + 1) * 512],
                             start=True, stop=True)
            hq = h_pool.tile([128, 512], bf16, name=f"hq{c}_{q}", tag="hq")
            nc.scalar.activation(hq[:], ht[:], AF.Tanh, bias=b1_t[:, 0:1])
            return hq

        def logit_mms(c, lg, hqs, t0, t1):
            for t in range(t0, t1):
                nc.tensor.matmul(lg[:, t:t + 1],
                                 lhsT=hqs[t // 4][:, (t % 4) * 128:(t % 4 + 1) * 128],
                                 rhs=w2_t[:], start=True, stop=True)

        def exp_stage(c, lg):
            ee = ee_pool.tile([128, TPC], bf16, name=f"ee{c}", tag="ee")
            nc.scalar.activation(ee[:], lg[:], AF.Exp, bias=b2_t[:, 0:1])
            return ee

        def me01_stage(c, sg):
            me01 = me01_pool.tile([128, TPC * MSLOT], bf16, name=f"m0{c}", tag="me01")
            sg_b = sg.unsqueeze(-1).broadcast_to([128, TPC, MSLOT])
            nc.vector.tensor_tensor(
                me01[:].rearrange("p (t s) -> p t s", t=TPC),
                iota_t[:].rearrange("p (t s) -> p t s", t=TPC),
                sg_b, op=ALU.is_equal)
            return me01

        def me_mult_stage(c, me01, ee):
            me = me_pool.tile([128, TPC * MSLOT], bf16, name=f"me{c}", tag="me")
            ee_b = ee[:].unsqueeze(-1).broadcast_to([128, TPC, MSLOT])
            nc.vector.tensor_tensor(
                me[:].rearrange("p (t s) -> p t s", t=TPC),
                me01[:].rearrange("p (t s) -> p t s", t=TPC),
                ee_b, op=ALU.mult)
            return me

        def pool_mms(c, pl, xn, me, t0, t1):
            strip = 64 * (c % 2)
            for t in range(t0, t1):
                nc.tensor.matmul(
                    pl[strip:strip + MSLOT, :],
                    lhsT=me[:, t * MSLOT:(t + 1) * MSLOT],
                    rhs=xn[:, t * (H + 1):(t + 1) * (H + 1)],
                    start=(t == 0), stop=(t == TPC - 1),
                    tile_position=(0, strip), skip_group_check=True)

        def pool_finish(c, pl):
            strip = 64 * (c % 2)
            nc.vector.tensor_copy(
                wres_cols[c // 2][strip:strip + MSLOT, :],
                pl[strip:strip + MSLOT, :])
            for Gi in ready_groups.get(c, ()):
                emit_stitch(Gi)

        # Three-stage software pipeline, interleaved per cycle. The logit
        # MMs of c-1 run at cycle START (their tanh finished last cycle),
        # so exp(c-1) fires mid-cycle and me(c-1) is ready well before its
        # pool MMs run in cycle c+1. Pool MMs of c-2 fill any PE gaps (the
        # scheduler runs whatever is ready, in emission-priority order),
        # and dense MLP MMs are spread across the cycle for HAM warmth.
        #   PE : [mm(c,q) | pool(c-2) x4 | (q<4) logit(c-1) x8] for q=0..7
        #   ACT: [exp(c-1) mid] [tanh(c) x8]
        #   DVE: [me01(c)] [me_mult(c-1)] [wres copy(c-2)]
        state = {}
        for c in range(NWIN):
            xt, xn, sg = chunk_setup(c)
            state[c] = dict(xn=xn, sg=sg)
            lg = lg_psum.tile([128, TPC], f32, name=f"lg{c - 1}", tag="lg") \
                if c >= 1 else None
            pl = pl_psum.tile([128, H + 1], f32, name=f"pl{c - 3}", tag="pl") \
                if c >= 3 else None
            hqs = []
            for q in range(8):
                hqs.append(mlp_mm(c, xt, q))
                if q == 0:
                    state[c]["me01"] = me01_stage(c, sg)
                if c >= 3:
                    pool_mms(c - 3, pl, state[c - 3]["xn"], state[c - 3]["me"],
                             4 * q, 4 * q + 4)
                if c >= 1 and q < 4:
                    logit_mms(c - 1, lg, state[c - 1]["hqs"],
                              8 * q, 8 * q + 8)
                if c >= 1 and q == 4:
                    ee = exp_stage(c - 1, lg)
                    state[c - 1]["me"] = me_mult_stage(
                        c - 1, state[c - 1]["me01"], ee)
                    state[c - 1]["hqs"] = None
            state[c]["hqs"] = hqs
            if c >= 3:
                pool_finish(c - 3, pl)

        # epilogue: logits/me for the last chunk, then the last two pools
        lg = lg_psum.tile([128, TPC], f32, name=f"lgE", tag="lg")
        logit_mms(NWIN - 1, lg, state[NWIN - 1]["hqs"], 0, TPC)
        ee = exp_stage(NWIN - 1, lg)
        state[NWIN - 1]["me"] = me_mult_stage(
            NWIN - 1, state[NWIN - 1]["me01"], ee)
        for c in (NWIN - 3, NWIN - 2, NWIN - 1):
            pl = pl_psum.tile([128, H + 1], f32, name=f"pl{c}", tag="pl")
            pool_mms(c, pl, state[c]["xn"], state[c]["me"], 0, TPC)
            pool_finish(c, pl)

    nc.compile()
    return nc


def kernel(x, batch, W1, b1, W2, b2):
    global LAST_EXEC_NS
    import os
    from concourse.bass_utils import run_bass_kernel_spmd

    x = np.asarray(x)
    batch = np.asarray(batch)
    shared, cores = _preprocess(x, batch, W1, b1, W2, b2, N_GRAPHS)

    key = (shared["NWIN"], shared["NGRP"], tuple(shared["wlo_shared"]))
    nc = _PROGRAM_CACHE.get(key)
    if nc is None:
        nc = _build_program(shared["NWIN"], shared["NGRP"], shared["wlo_shared"])
        _PROGRAM_CACHE[key] = nc

    in_maps = []
    for d in cores:
        in_maps.append({
            "xT": d["xt_img"], "xn": d["xn_img"], "seg": d["seg_img"],
            "iota": shared["iota"], "w1b": shared["w1b"], "w2b": shared["w2b"],
            "b1c": shared["b1c"], "b2c": shared["b2c"], "ind": d["ind_img"],
        })
    trace = os.environ.get("ATTNPOOL_TRACE", "0") == "1"
    res = run_bass_kernel_spmd(nc, in_maps, core_ids=list(range(N_CORES)),
                               trace=trace)
    if res.exec_time_ns is not None:
        LAST_EXEC_NS = res.exec_time_ns

    out = np.zeros((N_GRAPHS, H), dtype=np.float32)
    gsplit = shared["gsplit"]
    for c, d in enumerate(cores):
        out[gsplit[c]:gsplit[c + 1]] = res.results[c]["out"][:d["gc"]]
    out[shared["counts"] == 0] = 0.0
    return out
